# revision 1
# baseline (speedup 1.0000x reference)
"""Trainium2 Bass kernel for nn_DiscriminatorModel (8-layer MLP with
LayerNorm+LeakyReLU, 524288x128 input, data-parallel over 8 NeuronCores).

Algorithm (validated in numpy to ~7e-4 relative absmax vs the jax reference):
  - Mean-centering of each LayerNorm is folded into the weights host-side:
    Wc_l = W_l @ (I - 11^T/d)  => matmul output is already centered.
  - The per-row rsqrt(var+eps) scales commute through LeakyReLU and the
    following matmul, so they are never applied per-layer; only the variances
    of layers 6 and 7 matter to fp32 precision:
        E8 = v7 + eps*v6  (+O(eps^2)),  out = (a7 @ W8) / sqrt(E8) + b8
  - fp32-grade precision via fp16 multi-word matmuls (3 terms):
        q = Whi@ahi + Whi@alo + Wlo@ahi,  fp32 PSUM accumulate.
  - Activations are packed "feature-major": 128 partitions = c blocks x dout
    features, rows along the free dim. LeakyReLU+gamma runs as ONE ScalarE
    activation instruction per tile (Prelu, per-partition scale, alpha=0.2;
    note Lrelu ignores alpha on this table build - Prelu honors it)
    reading PSUM directly. hi/lo split on VectorE/GpSimd.

Requires all LayerNorm beta == 0 (true for the reference inputs); otherwise
falls back to a numpy forward pass.
"""

import numpy as np

EPS = 1e-5
SLOPE = 0.2
DIMS = [128, 32, 64, 32, 16, 8, 4, 2]
N_CORES = 8
ROWS = 524288
RPC = ROWS // N_CORES        # 65536 rows per core
R_ST = 8192                  # rows per supertile
N_ST = RPC // R_ST           # 8 supertiles per core
F16 = np.float16

_CACHE = {}


def _lrelu(x):
    return np.where(x > 0, x, SLOPE * x).astype(np.float32)


def _center(W):
    d = W.shape[1]
    return (W.astype(np.float64) @ (np.eye(d) - 1.0 / d)).astype(np.float32)


def _split(a):
    hi = a.astype(F16)
    lo = (a.astype(np.float32) - hi.astype(np.float32)).astype(F16)
    return hi, lo


def _blockdiag(W, c):
    din, dout = W.shape
    out = np.zeros((c * din, c * dout), W.dtype)
    for b in range(c):
        out[b * din:(b + 1) * din, b * dout:(b + 1) * dout] = W
    return out


def _transition_stat(W, c_in):
    """Parity-interleaved stationary for a c_in -> 2*c_in packing transition.

    Two stats (par=0,1), each [128, 128]: out col m = blk_out*w + f where
    w = 128/(2*c_in) per-block output width; nonzero iff blk_out % 2 == par,
    source block g = blk_out // 2 maps rows g*din..(g+1)*din <- W[:, f].
    """
    din, dout = W.shape
    w = 128 // (2 * c_in)
    assert w == dout
    stats = []
    for par in range(2):
        S = np.zeros((128, 128), W.dtype)
        for m in range(128):
            blk_out, f = divmod(m, w)
            if blk_out % 2 != par:
                continue
            g = blk_out // 2
            S[g * din:(g + 1) * din, m] = W[:, f]
        stats.append(S)
    return stats


def _var_stats(dout6, dout7):
    # V6 par-stats: s6 is 32-packed (32 blocks x 4 feats); v6' is 64 blocks.
    V6 = []
    for par in range(2):
        S = np.zeros((128, 64), np.float32)
        for m in range(64):
            if m % 2 != par:
                continue
            g = m // 2
            S[g * dout6:(g + 1) * dout6, m] = 1.0 / dout6
        V6.append(S)
    V7 = np.zeros((128, 64), np.float32)
    for m in range(64):
        V7[m * dout7:(m + 1) * dout7, m] = 1.0 / dout7
    return V6[0], V6[1], V7


def _numpy_forward(inp):
    h = np.asarray(inp["x"], np.float32)
    for i in range(7):
        W = np.asarray(inp[f"W{i+1}"], np.float32)
        g = np.asarray(inp[f"g{i+1}"], np.float32)
        b = np.asarray(inp[f"bt{i+1}"], np.float32)
        h = h @ W
        m = h.mean(-1, keepdims=True)
        v = np.square(h - m).mean(-1, keepdims=True)
        h = (h - m) / np.sqrt(v + EPS) * g + b
        h = _lrelu(h)
    return (h @ np.asarray(inp["W8"], np.float32)
            + np.asarray(inp["b8"], np.float32)).astype(np.float32)


def _build_consts(inp):
    """Host-side weight prep. Returns dict of DRAM const arrays."""
    Wc = [_center(np.asarray(inp[f"W{l}"], np.float32)) for l in range(1, 8)]
    g = [np.asarray(inp[f"g{l}"], np.float32) for l in range(1, 8)]
    W8 = np.asarray(inp["W8"], np.float32)

    cols = {}
    def add(name, arr32, pair=True):
        if pair:
            hi, lo = _split(arr32)
            cols[name + "h"], cols[name + "l"] = hi, lo
        else:
            cols[name] = arr32.astype(F16)

    add("s1", Wc[0])                                   # [128, 32]
    # L2 row-tiled: blockdiag2(Wc2) [64,128] stacked twice -> [128,128]
    bd2 = _blockdiag(Wc[1], 2)
    add("s2", np.vstack([bd2, bd2]))
    add("s3", _blockdiag(Wc[2], 2))                    # [128, 64]
    for l, c_in in ((4, 4), (5, 8), (6, 16), (7, 32)):
        t0, t1 = _transition_stat(Wc[l - 1], c_in)
        add(f"t{l}a", t0)
        add(f"t{l}b", t1)
    add("s8", _blockdiag(W8, 64))                      # [128, 64]

    # pack all fp16 stationaries into one [128, T] array; remember offsets
    order = sorted(cols.keys())
    offs, total = {}, 0
    for k in order:
        offs[k] = total
        total += cols[k].shape[1]
    wpack = np.zeros((128, total), F16)
    for k in order:
        wpack[:, offs[k]:offs[k] + cols[k].shape[1]] = cols[k]

    V6a, V6b, V7 = _var_stats(DIMS[6], DIMS[7])
    vpack = np.concatenate([V6a, V6b, V7], axis=1).astype(np.float32)

    # per-partition gamma vectors matching each layer's output packing
    gv = np.zeros((128, 8), np.float32)
    widths = [32, 64, 32, 16, 8, 4, 2]
    for i in range(7):
        gv[:, i] = np.tile(g[i], 128 // widths[i])
    return wpack, offs, gv, vpack



def _split_multi_waits(nc):
    """Walrus build limit: <=1 sync wait per instruction. Hoist extras onto
    same-engine NOPs inserted just before the instruction."""
    import concourse.mybir as mybir
    import bass_rust
    cnt = 0
    for f in nc.m.functions:
        for blk in f.blocks:
            newlist = []
            for inst in blk.instructions:
                si = inst.sync_info
                waits = list(si.on_wait) if si is not None and si.on_wait else []
                if len(waits) > 1:
                    for w in waits[:-1]:
                        nop = mybir.InstNoOp(name=f"waitnop_{cnt}", ins=[], outs=[])
                        cnt += 1
                        nop.engine = inst.engine
                        nop.sync_info = bass_rust.SyncInfo(on_wait=[w], on_update=[])
                        newlist.append(nop)
                    inst.sync_info = bass_rust.SyncInfo(
                        on_wait=[waits[-1]], on_update=list(si.on_update))
                newlist.append(inst)
            blk.instructions = newlist
    return cnt


def _build_program(offs, wpack_cols, b8_val):
    import concourse.bass as bass
    import concourse.mybir as mybir
    from concourse.tile import TileContext
    from contextlib import ExitStack

    # this walrus build rejects >1 sync wait on the tail Drain; split them
    import bass_rust
    from concourse.tile import TileContext as _TC
    from concourse.vector_clock import ScopedClock

    def _patched_drain(self, tick_clock, wait_clock):
        probe = self.nc.sync.nop()
        wait_clock.add_sem_waits(probe.ins,
                                 ScopedClock({None: tick_clock.global_clock}))
        si = probe.ins.sync_info
        waits = list(si.on_wait) if si is not None else []
        upd = list(si.on_update) if si is not None else []
        probe.ins.sync_info = bass_rust.SyncInfo(on_wait=waits[:1], on_update=upd)
        for w in waits[1:]:
            nop = self.nc.sync.nop()
            nop.ins.sync_info = bass_rust.SyncInfo(on_wait=[w], on_update=[])
        self.nc.sync.drain()
        self.nc.all_engine_barrier()
        assert self.sems is not None
        popped = self.nc._tile_sem_poison_stack.pop()
        assert popped is self._sem_poison
        self.nc.clear_and_free_semaphores(list(self.sems.allocated().values()))
        self.nc.all_engine_barrier()

    _TC._drain_and_barrier = _patched_drain

    f16, f32 = mybir.dt.float16, mybir.dt.float32
    AF = mybir.ActivationFunctionType
    OP = mybir.AluOpType

    nc = bass.Bass(trn_type="TRN2", num_swdge_queues=4)
    xhi_d = nc.dram_tensor("xhi", [128, RPC], f16, kind="ExternalInput")
    xlo_d = nc.dram_tensor("xlo", [128, RPC], f16, kind="ExternalInput")
    wp_d = nc.dram_tensor("wpack", [128, wpack_cols], f16, kind="ExternalInput")
    gv_d = nc.dram_tensor("gv", [128, 8], f32, kind="ExternalInput")
    vp_d = nc.dram_tensor("vpack", [128, 192], f32, kind="ExternalInput")
    out_d = nc.dram_tensor("out", [N_ST * 64, R_ST // 64], f32,
                           kind="ExternalOutput")

    with TileContext(nc) as tc:
        with ExitStack() as ctx:
            const = ctx.enter_context(tc.tile_pool(name="const", bufs=1))
            wp = const.tile([128, wpack_cols], f16)
            nc.sync.dma_start(wp[:, :], wp_d[:, :])
            gv = const.tile([128, 8], f32)
            nc.sync.dma_start(gv[:, :], gv_d[:, :])
            vpk = const.tile([128, 192], f32)
            nc.sync.dma_start(vpk[:, :], vp_d[:, :])

            def W(name):
                return wp[:, offs[name]:offs[name] + _WCOLS[name]]

            xp = ctx.enter_context(tc.tile_pool(name="xp", bufs=2))
            ap = ctx.enter_context(tc.tile_pool(name="ap", bufs=2))
            fin = ctx.enter_context(tc.tile_pool(name="fin", bufs=2 * N_ST))
            up = ctx.enter_context(tc.tile_pool(name="up", bufs=2, space="PSUM"))
            vp = ctx.enter_context(tc.tile_pool(name="vp", bufs=3, space="PSUM"))

            def mm(out, lhsT, rhs, start, stop, tp=None):
                # matmul output must fit one PSUM bank: 512 fp32 columns
                n = out.shape[1]
                for o in range(0, n, 512):
                    e = min(o + 512, n)
                    nc.tensor.matmul(out[:, o:e], lhsT, rhs[:, o:e],
                                     start=start, stop=stop, tile_position=tp)

            ysbs, e8sbs = [], []

            for st in range(N_ST):
                x0 = st * R_ST
                xh = []
                xl = []
                dma_engs = [nc.sync, nc.gpsimd, nc.scalar, nc.gpsimd]
                for k in range(2):
                    xht = xp.tile([128, 4096], f16, name=f"xh{k}")
                    dma_engs[2 * k].dma_start(
                        xht[:, :],
                        xhi_d[:, x0 + 4096 * k:x0 + 4096 * (k + 1)])
                    xh.append(xht)
                    xlt = xp.tile([128, 4096], f16, name=f"xl{k}")
                    dma_engs[2 * k + 1].dma_start(
                        xlt[:, :],
                        xlo_d[:, x0 + 4096 * k:x0 + 4096 * (k + 1)])
                    xl.append(xlt)

                def act_split(u, gcol, width, hi, lo, col0, eng):
                    """ACT Lrelu (PSUM->SBUF fp32), then hi/lo fp16 split."""
                    n = u.shape[1]
                    af = ap.tile([128, 1024], f32, name="af", tag="af", bufs=4)
                    afv = af[:, :n]
                    nc.scalar.activation(afv, u[:, :], AF.Prelu,
                                         bias=0.0, scale=gv[:, gcol:gcol + 1],
                                         alpha=SLOPE)
                    nc.vector.tensor_copy(hi[:, col0:col0 + n], afv)
                    eng.tensor_tensor(lo[:, col0:col0 + n], afv,
                                      hi[:, col0:col0 + n], OP.subtract)

                # ---- L1: u1 [128, 2048] (c=4), 2 psum chunks
                a1h = ap.tile([128, 2048], f16)
                a1l = ap.tile([128, 2048], f16)
                for c in range(2):
                    u = up.tile([128, 1024], f32, name="u", tag="u")
                    # term-outer order: adjacent mms hit different col-groups
                    # so their LDWEIGHTS overlap in-flight matmuls
                    for t in range(3):
                        for b in range(4):
                            rh = xh[b // 2][:, (b % 2) * 2048 + 1024 * c:][:, :1024]
                            rl = xl[b // 2][:, (b % 2) * 2048 + 1024 * c:][:, :1024]
                            S = W("s1h") if t < 2 else W("s1l")
                            r = rh if t != 1 else rl
                            mm(u[32 * b:32 * (b + 1), :], S, r,
                               start=(t == 0), stop=(t == 2), tp=(0, 32 * b))
                    act_split(u, 0, 32, a1h, a1l, 1024 * c, nc.vector)

                # ---- L2 row-tiled: two tensors u2_q [128, 2048]
                a2h = [ap.tile([128, 2048], f16, name=f"a2h{q}") for q in range(2)]
                a2l = [ap.tile([128, 2048], f16, name=f"a2l{q}") for q in range(2)]
                s2hs = [wp[:, offs["s2h"]:offs["s2h"] + 128][64 * q:64 * (q + 1), :]
                        for q in range(2)]
                s2ls = [wp[:, offs["s2l"]:offs["s2l"] + 128][64 * q:64 * (q + 1), :]
                        for q in range(2)]
                for c in range(2):
                    us = [up.tile([128, 1024], f32, name="u", tag="u")
                          for _ in range(2)]
                    for t in range(3):
                        for q in range(2):
                            rh = a1h[64 * q:64 * (q + 1), 1024 * c:1024 * (c + 1)]
                            rl = a1l[64 * q:64 * (q + 1), 1024 * c:1024 * (c + 1)]
                            S = s2hs[q] if t < 2 else s2ls[q]
                            r = rh if t != 1 else rl
                            mm(us[q][:, :], S, r, start=(t == 0),
                               stop=(t == 2), tp=(64 * q, 0))
                    for q in range(2):
                        act_split(us[q], 1, 64, a2h[q], a2l[q], 1024 * c,
                                  nc.vector)

                # ---- L3 col-tiled: u3 [128, 2048] (c=4)
                a3h = ap.tile([128, 2048], f16)
                a3l = ap.tile([128, 2048], f16)
                for c in range(2):
                    u = up.tile([128, 1024], f32, name="u", tag="u")
                    for t in range(3):
                        for q in range(2):
                            rh = a2h[q][:, 1024 * c:1024 * (c + 1)]
                            rl = a2l[q][:, 1024 * c:1024 * (c + 1)]
                            S = W("s3h") if t < 2 else W("s3l")
                            r = rh if t != 1 else rl
                            mm(u[64 * q:64 * (q + 1), :], S, r,
                               start=(t == 0), stop=(t == 2), tp=(0, 64 * q))
                    act_split(u, 2, 32, a3h, a3l, 1024 * c, nc.vector)

                # ---- L4..L7: parity transitions, halving free size
                prev_h, prev_l = a3h, a3l
                n_prev = 2048
                s6 = s7 = None
                for li, l in enumerate((4, 5, 6, 7)):
                    n = n_prev // 2
                    u = up.tile([128, 1024], f32, name="u", tag="u")
                    uv = u[:, :n]
                    first = True
                    for par, suf in ((0, "a"), (1, "b")):
                        rh = prev_h[:, par * n:(par + 1) * n]
                        rl = prev_l[:, par * n:(par + 1) * n]
                        th, tl = W(f"t{l}{suf}h"), W(f"t{l}{suf}l")
                        mm(uv, th, rh, start=first, stop=False)
                        mm(uv, th, rl, start=False, stop=False)
                        mm(uv, tl, rh, start=False,
                                         stop=(par == 1))
                        first = False
                    nh = ap.tile([128, n], f16, name=f"a{l}h")
                    nl = ap.tile([128, n], f16, name=f"a{l}l")
                    if l == 6:
                        s6 = ap.tile([128, 256], f32)
                        nc.scalar.activation(s6[:, :], uv, AF.Square)
                    if l == 7:
                        s7 = ap.tile([128, 128], f32)
                        nc.scalar.activation(s7[:, :], uv, AF.Square)
                    act_split(u[:, :n], l - 1, 128 // (2 ** (li + 3)), nh, nl,
                              0, nc.vector)
                    prev_h, prev_l, n_prev = nh, nl, n

                # ---- variances
                v6t = vp.tile([64, 128], f32, name="v6t", tag="v")
                mm(v6t[:, :], vpk[:, 0:64], s6[:, 0:128],
                   start=True, stop=False)
                mm(v6t[:, :], vpk[:, 64:128], s6[:, 128:256],
                   start=False, stop=True)
                v7t = vp.tile([64, 128], f32, name="v7t", tag="v")
                mm(v7t[:, :], vpk[:, 128:192], s7[:, :],
                   start=True, stop=True)

                # ---- L8: y = a7 @ blockdiag64(W8)
                yt = vp.tile([64, 128], f32, name="yt", tag="v")
                mm(yt[:, :], W("s8h"), prev_h[:, :],
                                 start=True, stop=False)
                mm(yt[:, :], W("s8h"), prev_l[:, :],
                                 start=False, stop=False)
                mm(yt[:, :], W("s8l"), prev_h[:, :],
                                 start=False, stop=True)

                # ---- stash y and E8 = v7 + eps*v6 (sqrt deferred)
                v7sb = fin.tile([64, 128], f32, name="v7sb", tag="v7sb")
                nc.scalar.copy(v7sb[:, :], v7t[:, :])
                ysb = fin.tile([64, 128], f32, name="ysb", tag="ysb")
                nc.scalar.copy(ysb[:, :], yt[:, :])
                e8 = fin.tile([64, 128], f32, name="e8", tag="e8")
                nc.vector.scalar_tensor_tensor(e8[:, :], v6t[:, :], EPS,
                                               v7sb[:, :], OP.mult, OP.add)
                ysbs.append(ysb)
                e8sbs.append(e8)

            # ---- final: out = y / sqrt(E8) + b8 (one Sqrt table-load)
            for st in range(N_ST):
                sq = fin.tile([64, 128], f32, name="sq", tag="sq", bufs=2)
                nc.scalar.activation(sq[:, :], e8sbs[st][:, :], AF.Sqrt)
                rinv = fin.tile([64, 128], f32, name="rinv", tag="rinv", bufs=2)
                nc.vector.reciprocal(rinv[:, :], sq[:, :])
                osb = fin.tile([64, 128], f32, name="osb", tag="osb", bufs=2)
                nc.vector.tensor_tensor(osb[:, :], ysbs[st][:, :], rinv[:, :],
                                        OP.mult)
                nc.vector.tensor_scalar(osb[:, :], osb[:, :], b8_val,
                                        None, OP.add)
                nc.sync.dma_start(out_d[st * 64:(st + 1) * 64, :], osb[:, :])
    _split_multi_waits(nc)
    return nc


_WCOLS = {}


def kernel(**inputs):
    for l in range(1, 8):
        if np.abs(np.asarray(inputs[f"bt{l}"], np.float32)).max() > 0:
            return _numpy_forward(inputs)

    wpack, offs, gv, vpack = _build_consts(inputs)
    global _WCOLS
    _WCOLS = {"s1h": 32, "s1l": 32, "s2h": 128, "s2l": 128, "s3h": 64,
              "s3l": 64, "s8h": 64, "s8l": 64, "v6a": 64, "v6b": 64, "v7": 64}
    for l in range(4, 8):
        for suf in ("a", "b"):
            _WCOLS[f"t{l}{suf}h"] = 128
            _WCOLS[f"t{l}{suf}l"] = 128

    x = np.asarray(inputs["x"], np.float32)
    xT = np.ascontiguousarray(x.T)               # [128, 524288]
    xhi = xT.astype(F16)
    xlo = (xT - xhi.astype(np.float32)).astype(F16)
    b8 = np.asarray(inputs["b8"], np.float32).reshape(1, 1)

    nc = _build_program(offs, wpack.shape[1], float(b8[0, 0]))

    in_maps = []
    for c in range(N_CORES):
        s = slice(c * RPC, (c + 1) * RPC)
        in_maps.append({
            "xhi": np.ascontiguousarray(xhi[:, s]),
            "xlo": np.ascontiguousarray(xlo[:, s]),
            "wpack": wpack, "gv": gv, "vpack": vpack,
        })

    from concourse.bass_utils import run_bass_kernel_spmd
    res = run_bass_kernel_spmd(nc, in_maps, core_ids=list(range(N_CORES)))

    out = np.empty((ROWS, 1), np.float32)
    for c in range(N_CORES):
        out[c * RPC:(c + 1) * RPC, 0] = res.results[c]["out"].reshape(-1)
    return out

